# revision 1
# baseline (speedup 1.0000x reference)
"""Trainium2 Bass kernel for windowed multi-head attention with a dynamic
position-bias MLP (CrossFormer-style), data-parallel over windows on 8 cores.

Math per window (N=256 tokens, C=512 dim, H=8 heads, hd=64):
    qkv = x @ qkv_w + qkv_b ; q scaled by hd**-0.5
    attn = softmax(q @ k^T + rpb) ; out = (attn @ v) @ proj_w + proj_b
where rpb[h] = pos[rel_idx] and pos = MLP(biases) is a tiny 4-layer MLP
(LayerNorm + ReLU) applied to the 961 relative-offset rows, shared by all
windows.

v2 layout strategy on each NeuronCore (32 windows/core, processed in pairs):
  - x of a window PAIR is PE-transposed to x^T (channels on partitions,
    512 token columns) so QKV matmuls run with free size 512.
  - q^T/k^T are produced channel-major as head-pair tiles [128, 512].
  - v is evacuated STRIDED into vplus [128, 8, 65]: per head 64 value
    columns plus a ones column, so the PV matmul's stationary [128, 65]
    yields O^T rows 0..63 AND the softmax denominator in row 64 of the
    same PSUM tile -- no separate denominator matmuls.
  - S^T = k^T.T @ q^T per head with 2-head row-packing (K=64); rpb^T
    (pre-scaled by 1/scale, bf16) is accumulated into the same PSUM tile
    via an identity matmul; one ACT exp evacuates P^T (no max-subtraction:
    logits are O(1) by construction).
  - PV per head-pair packs into one PSUM bank [65, 512]; denominators of
    both heads share row 64 -> one DVE reciprocal [1,512], one gpsimd
    broadcast [64,512], one DVE multiply [64,512] per pair.
  - proj contracts over c in 8 chunks of 64 (stationary = oT pair slices).

All matmuls run as float32r (full fp32 data; ~1e-3 matmul rounding, 4x
faster than fp32 mode on the PE).
"""

import os
import sys

if "axon" not in os.environ.get("JAX_PLATFORMS", ""):
    os.environ["JAX_PLATFORMS"] = "axon"

for _p in (
    "/root/.axon_site",
    "/root/.axon_site/_ro/trn_rl_repo",
    "/root/.axon_site/_ro/pypackages",
    "/opt/trn_rl_repo",
):
    if os.path.isdir(_p) and _p not in sys.path:
        sys.path.append(_p)

import numpy as np

import concourse.bass as bass
import concourse.bacc as bacc
import concourse.mybir as mybir
import concourse.tile as tile
from concourse.bass_utils import run_bass_kernel_spmd

F32 = mybir.dt.float32
F32R = mybir.dt.float32r
BF16 = mybir.dt.bfloat16
EXP = mybir.ActivationFunctionType.Exp
SQRT = mybir.ActivationFunctionType.Sqrt
COPY = mybir.ActivationFunctionType.Copy
ADD = mybir.AluOpType.add
SUB = mybir.AluOpType.subtract
MULT = mybir.AluOpType.mult
MAX = mybir.AluOpType.max

B, N, C = 256, 256, 512
H, HD = 8, 64
PD, L = 32, 961  # pos-MLP width, (2*16-1)**2 offset rows
GH = GW = 16
NCORES = 8
WPC = B // NCORES  # windows per core
SCALE = HD ** -0.5
LN_EPS = 1e-5


def _host_consts():
    ident = np.eye(128, dtype=np.float32)
    sigma = np.array([(i // 16) * 16 + (15 - i % 16) for i in range(128)])
    sigperm = np.zeros((128, 128), np.float32)
    for i in range(128):
        sigperm[i, sigma[i]] = 1.0
    ones = np.ones((128, 128), np.float32)
    return ident, sigperm, ones


def build_program(wpc=WPC, repeat=1, has_qkv_b=True, has_proj_b=True, ablate=None):
    """Build the SPMD Bass program for one core handling `wpc` windows.

    repeat>1 wraps the steady-state pair loop in a hardware For loop for
    wall-clock timing (the computation is idempotent)."""
    assert wpc % 2 == 0
    nc = bacc.Bacc("TRN2", num_devices=NCORES)
    T = wpc * N  # tokens per core
    NP = 2 * N   # tokens per window pair

    x_d = nc.dram_tensor("x", [T, C], F32R, kind="ExternalInput")
    qkvw_d = nc.dram_tensor("qkv_w", [C, 3 * C], F32R, kind="ExternalInput")
    qkvb_d = nc.dram_tensor("qkv_b", [3 * C], F32, kind="ExternalInput")
    projw_d = nc.dram_tensor("proj_w", [C, C], F32R, kind="ExternalInput")
    projb_d = nc.dram_tensor("proj_b", [C], F32, kind="ExternalInput")
    pw_d = [
        nc.dram_tensor("p1_w", [2, PD], F32R, kind="ExternalInput"),
        nc.dram_tensor("p2_w", [PD, PD], F32R, kind="ExternalInput"),
        nc.dram_tensor("p3_w", [PD, PD], F32R, kind="ExternalInput"),
    ]
    pb_d = [
        nc.dram_tensor("p1_b", [PD], F32, kind="ExternalInput"),
        nc.dram_tensor("p2_b", [PD], F32, kind="ExternalInput"),
        nc.dram_tensor("p3_b", [PD], F32, kind="ExternalInput"),
    ]
    g_d = [
        nc.dram_tensor("g1", [PD], F32, kind="ExternalInput"),
        nc.dram_tensor("g2", [PD], F32, kind="ExternalInput"),
        nc.dram_tensor("g3", [PD], F32, kind="ExternalInput"),
    ]
    bln_d = [
        nc.dram_tensor("b1", [PD], F32, kind="ExternalInput"),
        nc.dram_tensor("b2", [PD], F32, kind="ExternalInput"),
        nc.dram_tensor("b3", [PD], F32, kind="ExternalInput"),
    ]
    p4w_d = nc.dram_tensor("p4_w", [PD, H], F32R, kind="ExternalInput")
    p4b_d = nc.dram_tensor("p4_b", [H], F32, kind="ExternalInput")
    biases_d = nc.dram_tensor("biases", [L, 2], F32R, kind="ExternalInput")
    ident_d = nc.dram_tensor("ident", [128, 128], F32R, kind="ExternalInput")
    sigperm_d = nc.dram_tensor("sigperm", [128, 128], F32R, kind="ExternalInput")
    ones_d = nc.dram_tensor("ones", [128, 128], F32R, kind="ExternalInput")
    y_d = nc.dram_tensor("y", [T, C], F32, kind="ExternalOutput")
    # per-core scratch holding pos^T rows, head-major [H*961]
    posdram = nc.dram_tensor("posdram", [H * L], F32)

    with tile.TileContext(nc) as tc:
        nc._allow_low_precision_reason = "float32r rounding of matmul operands is intended"
        from contextlib import ExitStack
        mlp_ctx = ExitStack()
        win_ctx = ExitStack()
        with (
            tc.tile_pool(name="const", bufs=1) as constp,
            tc.tile_pool(name="rpb", bufs=1) as rpbp,
            tc.tile_pool(name="psA", bufs=2, space="PSUM") as psA,
        ):
            # ---------------- Phase A: constants ----------------
            ident = constp.tile([128, 128], F32R)
            nc.scalar.dma_start(ident[:], ident_d[:])
            identb = constp.tile([128, 128], BF16)
            nc.scalar.activation(identb[:], ident[:].bitcast(F32), COPY)
            sigperm = constp.tile([128, 128], F32R)
            nc.scalar.dma_start(sigperm[:], sigperm_d[:])
            ones = constp.tile([128, 128], F32R)
            nc.scalar.dma_start(ones[:], ones_d[:])

            qw = []
            for k in range(4):
                t = constp.tile([128, 3 * C], F32R, tag=f"qw{k}")
                nc.scalar.dma_start(t[:], qkvw_d[k * 128:(k + 1) * 128, :])
                qw.append(t)
            pw128 = []
            for k in range(4):
                t = constp.tile([128, C], F32R, tag=f"pw{k}")
                nc.scalar.dma_start(t[:], projw_d[k * 128:(k + 1) * 128, :])
                pw128.append(t)

            fakeqk, fakevp = None, None
            if ablate == 'attnonly':
                fakeqk = []
                for mi in range(8):
                    t = constp.tile([128, 2 * N], F32R, tag=f'fqk{mi}')
                    nc.gpsimd.memset(t[:].bitcast(F32), 0.01)
                    fakeqk.append(t)
                fakevp = []
                for i in range(4):
                    t = constp.tile([128, H, HD + 2], F32R, tag=f'fvp{i}')
                    nc.gpsimd.memset(t[:].bitcast(F32), 1.0)
                    fakevp.append(t)

            # q/k bias columns: qbT[p, j] = qkv_b[j*128 + p], j in 0..7
            qbT = constp.tile([128, 8], F32)
            nc.sync.dma_start(
                qbT[:], bass.AP(tensor=qkvb_d[:].tensor, offset=0, ap=[[1, 128], [128, 8]])
            )
            # v bias broadcast [1,512] -> [128,512]
            vb1 = constp.tile([1, C], F32)
            nc.sync.dma_start(vb1[:], qkvb_d[2 * C:3 * C].unsqueeze(0))
            vb_bc = constp.tile([128, C], F32)
            nc.gpsimd.partition_broadcast(vb_bc[:], vb1[:])
            # proj bias broadcast
            pb1 = constp.tile([1, C], F32)
            nc.sync.dma_start(pb1[:], projb_d[:].unsqueeze(0))
            pb_bc = constp.tile([128, C], F32)
            nc.gpsimd.partition_broadcast(pb_bc[:], pb1[:])
            eps_ap = constp.tile([PD, 1], F32)
            nc.gpsimd.memset(eps_ap[:], LN_EPS)

            mlpp = mlp_ctx.enter_context(tc.tile_pool(name="mlp", bufs=1))
            psPro = mlp_ctx.enter_context(tc.tile_pool(name="psPro", bufs=2, space="PSUM"))
            # small MLP params
            pw_sb, pb_sb, g_sb, bln_sb = [], [], [], []
            for i in range(3):
                wt = mlpp.tile(list(pw_d[i].shape), F32R, tag=f"pw_sb{i}")
                nc.sync.dma_start(wt[:], pw_d[i][:])
                pw_sb.append(wt)
                bt = mlpp.tile([PD, 1], F32, tag=f"pb_sb{i}")
                nc.sync.dma_start(bt[:], pb_d[i][:].unsqueeze(1))
                pb_sb.append(bt)
                gt = mlpp.tile([PD, 1], F32, tag=f"g_sb{i}")
                nc.sync.dma_start(gt[:], g_d[i][:].unsqueeze(1))
                g_sb.append(gt)
                lt = mlpp.tile([PD, 1], F32, tag=f"bln_sb{i}")
                nc.sync.dma_start(lt[:], bln_d[i][:].unsqueeze(1))
                bln_sb.append(lt)
            p4w_sb = mlpp.tile([PD, H], F32R)
            nc.sync.dma_start(p4w_sb[:], p4w_d[:])
            p4b_sb = mlpp.tile([H, 1], F32)
            nc.sync.dma_start(p4b_sb[:], p4b_d[:].unsqueeze(1))

            # biases -> biasesT [2, 961] via PE transposes of [128,2] tiles
            biasesT = mlpp.tile([2, L], F32R)
            for i in range(8):
                rows = min(128, L - i * 128)
                rpad = rows + (rows % 2)
                bt = mlpp.tile([128, 2], F32R, tag="btile")
                if rpad != rows:
                    nc.gpsimd.memset(bt[:].bitcast(F32), 0.0)
                nc.sync.dma_start(bt[0:rows, :], biases_d[i * 128:i * 128 + rows, :])
                tp = psPro.tile([2, 128], F32, tag="trp")
                nc.tensor.transpose(tp[:, 0:rpad].bitcast(F32R), bt[0:rpad, :], ident[0:rpad, 0:rpad])
                nc.scalar.copy(biasesT[:, i * 128:i * 128 + rows], tp[:, 0:rows])

            # ---------------- Phase B: pos MLP (feature-on-partition) ----------
            segs = [(0, 512), (L - 512, 512)]  # overlap keeps fp32r free-size even
            h_cur = biasesT
            ln_scale = 1.0 / PD
            for li in range(3):
                kdim = 2 if li == 0 else PD
                z = mlpp.tile([PD, L], F32R, tag="z", bufs=1)
                xm = mlpp.tile([PD, L], F32R, tag="xm", bufs=1)
                sq = mlpp.tile([PD, L], F32R, tag="sq", bufs=1)
                mean = mlpp.tile([1, L], F32R, tag="mean", bufs=2)
                sd = mlpp.tile([1, L], F32, tag="sd", bufs=2)
                rstd = mlpp.tile([1, L], F32R, tag="rstd", bufs=2)
                hn = mlpp.tile([PD, L], F32R, tag=f"h{li % 2}", bufs=1)
                for s0, sl in segs:
                    zp = psA.tile([PD, 512], F32, tag="A")
                    nc.tensor.matmul(zp[:, 0:sl], pw_sb[li][0:kdim, :], h_cur[0:kdim, s0:s0 + sl],
                                     start=True, stop=True)
                    nc.vector.tensor_scalar(z[:, s0:s0 + sl], zp[:, 0:sl], pb_sb[li][:], None, op0=ADD)
                    mp = psPro.tile([1, 512], F32, tag="dn")
                    nc.tensor.matmul(mp[0:1, 0:sl], ones[0:PD, 0:1], z[:, s0:s0 + sl].bitcast(F32R),
                                     start=True, stop=True)
                    nc.scalar.activation(mean[:, s0:s0 + sl], mp[0:1, 0:sl], COPY, scale=ln_scale)
                    mb = psPro.tile([PD, 512], F32, tag="trp")
                    nc.tensor.matmul(mb[:, 0:sl], ones[0:1, 0:PD], mean[:, s0:s0 + sl],
                                     start=True, stop=True)
                    nc.vector.tensor_tensor(xm[:, s0:s0 + sl], z[:, s0:s0 + sl], mb[:, 0:sl], op=SUB)
                    nc.vector.tensor_tensor(sq[:, s0:s0 + sl], xm[:, s0:s0 + sl], xm[:, s0:s0 + sl], op=MULT)
                    vp = psPro.tile([1, 512], F32, tag="dn")
                    nc.tensor.matmul(vp[0:1, 0:sl], ones[0:PD, 0:1], sq[:, s0:s0 + sl],
                                     start=True, stop=True)
                    nc.scalar.activation(sd[:, s0:s0 + sl], vp[0:1, 0:sl], SQRT,
                                         bias=eps_ap[0:1, :], scale=ln_scale)
                    nc.vector.reciprocal(rstd[:, s0:s0 + sl], sd[:, s0:s0 + sl])
                    rb = psPro.tile([PD, 512], F32, tag="trp")
                    nc.tensor.matmul(rb[:, 0:sl], ones[0:1, 0:PD], rstd[:, s0:s0 + sl],
                                     start=True, stop=True)
                    nc.vector.tensor_tensor(hn[:, s0:s0 + sl], xm[:, s0:s0 + sl], rb[:, 0:sl], op=MULT)
                    # gamma * h + beta, then relu
                    nc.vector.tensor_scalar(hn[:, s0:s0 + sl], hn[:, s0:s0 + sl],
                                            g_sb[li][:], bln_sb[li][:], op0=MULT, op1=ADD)
                    nc.vector.tensor_scalar(hn[:, s0:s0 + sl], hn[:, s0:s0 + sl], 0.0, None, op0=MAX)
                h_cur = hn

            posT = mlpp.tile([H, L], F32)
            for s0, sl in segs:
                pp = psA.tile([H, 512], F32, tag="A")
                nc.tensor.matmul(pp[:, 0:sl], p4w_sb[:], h_cur[:, s0:s0 + sl], start=True, stop=True)
                # pre-scale by 1/SCALE: the window-loop exp applies scale to S+rpb
                nc.vector.tensor_scalar(posT[:, s0:s0 + sl], pp[:, 0:sl], p4b_sb[:], 1.0 / SCALE,
                                        op0=ADD, op1=MULT)
            nc.sync.dma_start(
                bass.AP(tensor=posdram[:].tensor, offset=0, ap=[[L, H], [1, L]]), posT[:]
            )

            # ------------- Phase C: rpb^T tiles [128, 512] per head ----------
            # sigma-ordered gather (the DMA-legal order), then a permutation
            # transpose + plain transpose per 128-column half to undo sigma.
            # Tile h holds both m-chunks side by side: cols [mc*256, mc*256+256).
            rpbT = [rpbp.tile([128, 2 * N], BF16, tag=f"rpb{h}", name=f"rpb{h}") for h in range(H)]
            for h in range(H):
                for c in range(2):
                    sig = rpbp.tile([128, N], F32, tag="rpbsig")
                    for mhl in range(8):
                        mh = c * 8 + mhl
                        src = bass.AP(tensor=posdram[:].tensor,
                                      offset=h * L + (15 - mh) * 31,
                                      ap=[[1, 16], [31, 16], [1, 16]])
                        nc.scalar.dma_start(
                            sig[mhl * 16:(mhl + 1) * 16, :].rearrange("p (a b) -> p a b", b=16), src
                        )
                    for half in range(2):
                        t1 = psPro.tile([128, 128], F32, tag="trp")
                        nc.tensor.matmul(t1[:], sig[:, half * 128:(half + 1) * 128],
                                         sigperm[:].bitcast(F32), is_transpose=True)
                        tmp = rpbp.tile([128, 128], F32, tag="rpbtmp")
                        nc.scalar.copy(tmp[:], t1[:])
                        t2 = psPro.tile([128, 128], F32, tag="trp")
                        nc.tensor.transpose(t2[:], tmp[:], ident[:].bitcast(F32))
                        nc.vector.tensor_copy(
                            rpbT[h][:, c * N + half * 128: c * N + (half + 1) * 128], t2[:])

            mlp_ctx.close()  # free MLP SBUF before the window loop
            winp = win_ctx.enter_context(tc.tile_pool(name="win", bufs=1))
            psS = win_ctx.enter_context(tc.tile_pool(name="psS", bufs=3, space="PSUM"))
            psPV = win_ctx.enter_context(tc.tile_pool(name="psPV", bufs=2, space="PSUM"))
            psY = win_ctx.enter_context(tc.tile_pool(name="psY", bufs=1, space="PSUM"))

            # ---------------- Phase D: window-pair loop ----------------
            def qkv_stage(p, st):
                # generator: yields between chunks so the driver can
                # interleave the previous pair's attention into the stream
                w0 = 2 * p  # first window of pair; tokens [w0*N, w0*N + 512)
                # -- load x [4 x [128, 512]] --
                xa = []
                for c in range(4):
                    t = winp.tile([128, C], F32R, tag=f"xa{c}", bufs=2)
                    nc.sync.dma_start(t[:], x_d[w0 * N + c * 128: w0 * N + (c + 1) * 128, :])
                    xa.append(t)
                # -- x^T: channels on partitions, 512 pair-tokens on free --
                xT = []
                for k in range(4):
                    tp = psA.tile([128, NP], F32, tag="A")
                    for c in range(4):
                        nc.tensor.transpose(tp[:, c * 128:(c + 1) * 128].bitcast(F32R),
                                            xa[c][:, k * 128:(k + 1) * 128], ident[:])
                    t = winp.tile([128, NP], F32R, tag=f"xT{k}", bufs=2)
                    nc.scalar.copy(t[:], tp[:])
                    xT.append(t)
                yield
                # -- q^T / k^T head-pair tiles [128, 512] (mi 0..3 = q, 4..7 = k) --
                qkT = []
                for mi in range(8):
                    ps = psA.tile([128, NP], F32, tag="A")
                    for k in range(4):
                        nc.tensor.matmul(ps[:], qw[k][:, mi * 128:(mi + 1) * 128], xT[k][:],
                                         start=(k == 0), stop=(k == 3))
                    t = winp.tile([128, NP], F32R, tag=f"qkT{mi}", bufs=2)
                    if has_qkv_b:
                        nc.vector.tensor_scalar(t[:], ps[:], qbT[:, mi:mi + 1], None, op0=ADD)
                    else:
                        nc.scalar.copy(t[:], ps[:])
                    qkT.append(t)
                    if mi == 3:
                        yield
                st["qkT"] = qkT
                yield
                # -- v, evacuated strided into vplus [128, 8, 65] (ones in col 64) --
                vplus = []  # [w][mc] -> [128, 8, 65]
                for w in range(2):
                    vrow = []
                    for mc in range(2):
                        ps = psA.tile([128, C], F32, tag="A")
                        for k in range(4):
                            nc.tensor.matmul(
                                ps[:], xT[k][:, w * N + mc * 128: w * N + (mc + 1) * 128],
                                qw[k][:, 2 * C:3 * C], start=(k == 0), stop=(k == 3))
                        t = winp.tile([128, H, HD + 2], F32R, tag=f"vp{w}{mc}", bufs=2)
                        nc.gpsimd.memset(t[:, :, HD:HD + 2].bitcast(F32), 1.0)
                        src3 = ps[:].rearrange("p (h e) -> p h e", e=HD)
                        if has_qkv_b:
                            vb3 = vb_bc[:].rearrange("p (h e) -> p h e", e=HD)
                            nc.vector.tensor_tensor(t[:, :, 0:HD], src3, vb3, op=ADD)
                        else:
                            nc.vector.tensor_copy(t[:, :, 0:HD], src3)
                        vrow.append(t)
                    vplus.append(vrow)
                st["vplus"] = vplus

            # proj of a window is DEFERRED to the next pair's QKV stage and
            # the two windows of a pair are fully INTERLEAVED so every engine
            # always has independent ready work (cross-engine sem latency on
            # HW is far larger than modeled; chains must be double-booked).
            pending_proj = []

            def flush_proj():
                for f in pending_proj:
                    f()
                del pending_proj[:]

            def run_attention(w0, qkT, vplus):
                flush_proj()
                yield
                pT = [[None] * H, [None] * H]
                oSb = [[None, None], [None, None]]

                def do_S(w, h):
                    tw = w * N
                    bp = (h % 2) * 64
                    kq = qkT[4 + h // 2]
                    qq = qkT[h // 2]
                    sps = psS.tile([128, 2 * N], F32, tag="S", name=f"sps{w}{h}")
                    for mc in range(2):
                        nc.tensor.matmul(
                            sps[:, mc * N:(mc + 1) * N],
                            kq[bp:bp + 64, tw + mc * 128: tw + (mc + 1) * 128],
                            qq[bp:bp + 64, tw:tw + N],
                            start=(mc == 0), stop=False,
                            tile_position=(bp, 0))
                    # += rpb^T (identity matmul closes the accumulation group)
                    nc.tensor.matmul(sps[:], identb[:], rpbT[h][:], start=False, stop=True)
                    t = winp.tile([128, 2 * N], F32R, tag="pT", bufs=13, name=f"pT{w}{h}")
                    nc.scalar.activation(t[:], sps[:], EXP, scale=SCALE)
                    pT[w][h] = t

                def do_PV(w, j):
                    # token-major PV: out[t, c_h | dn_h]; j = (tchunk, headgroup)
                    tc, hg = j % 2, j // 2
                    ops = psPV.tile([128, 4 * (HD + 2)], F32, tag="PV", name=f"ops{w}{j}")
                    for i in range(4):
                        h = 4 * hg + i
                        for mc in range(2):
                            nc.tensor.matmul(
                                ops[:, i * (HD + 2):(i + 1) * (HD + 2)],
                                pT[w][h][:, mc * N + tc * 128: mc * N + (tc + 1) * 128],
                                vplus[w][mc][:, h, :],
                                start=(mc == 0), stop=(mc == 1))
                    if oSb[w][tc] is None:
                        oSb[w][tc] = winp.tile([128, C], F32R, tag=f"oSb{tc}", bufs=3,
                                               name=f"oSb{w}{tc}")
                    ops3 = ops[:].rearrange("p (h e) -> p h e", e=HD + 2)
                    if ablate == 'fastevac':
                        nc.vector.tensor_copy(
                            oSb[w][tc][:, 4 * hg * HD:(4 * hg + 4) * HD], ops[:, 0:4 * HD])
                        return
                    rcpt = winp.tile([128, 4], F32, tag="rcpt", bufs=6)
                    nc.vector.reciprocal(
                        rcpt[:].rearrange("p (h e) -> p h e", e=1), ops3[:, :, HD:HD + 1])
                    for i in range(4):
                        h = 4 * hg + i
                        nc.vector.tensor_scalar(
                            oSb[w][tc][:, h * HD:(h + 1) * HD],
                            ops[:, i * (HD + 2):i * (HD + 2) + HD],
                            rcpt[:, i:i + 1], None, op0=MULT)

                def mk_proj(w):
                    myoSb = oSb[w]

                    def do_proj():
                        # O^T via PE transposes, then proj with K=128
                        ochan = winp.tile([128, 2 * C], F32R, tag="ochan", bufs=2,
                                          name=f"ochan{w}")
                        for tc in range(2):
                            tp = psA.tile([128, C], F32, tag="A")
                            for cb in range(4):
                                nc.tensor.transpose(
                                    tp[:, cb * 128:(cb + 1) * 128].bitcast(F32R),
                                    myoSb[tc][:, cb * 128:(cb + 1) * 128], ident[:])
                            ochan3 = ochan[:].rearrange("p (cb t) -> p cb t", t=2 * 128)
                            nc.scalar.copy(ochan3[:, :, tc * 128:(tc + 1) * 128],
                                           tp[:].rearrange("p (c e) -> p c e", e=128))
                        for tc in range(2):
                            ps = psY.tile([128, C], F32, tag="Y")
                            for cb in range(4):
                                nc.tensor.matmul(
                                    ps[:], ochan[:, cb * 256 + tc * 128: cb * 256 + (tc + 1) * 128],
                                    pw128[cb][:], start=(cb == 0), stop=(cb == 3))
                            yo = winp.tile([128, C], F32, tag=f"yo{tc}", bufs=2)
                            if has_proj_b:
                                nc.vector.tensor_tensor(yo[:], ps[:], pb_bc[:], op=ADD)
                            else:
                                nc.vector.tensor_copy(yo[:], ps[:])
                            nc.scalar.dma_start(
                                y_d[(w0 + w) * N + tc * 128: (w0 + w) * N + (tc + 1) * 128, :], yo[:])
                    return do_proj

                for h in range(4):
                    do_S(0, h)
                yield
                for h in range(4):
                    do_S(1, h)
                if ablate == 'sonly':
                    for w in range(2):
                        for h in range(4, H):
                            do_S(w, h)
                    return
                yield
                do_PV(0, 0)
                do_PV(0, 1)
                for h in range(4, H):
                    do_S(0, h)
                yield
                do_PV(1, 0)
                do_PV(1, 1)
                for h in range(4, H):
                    do_S(1, h)
                yield
                do_PV(0, 2)
                do_PV(0, 3)
                yield
                do_PV(1, 2)
                do_PV(1, 3)
                pending_proj.append(mk_proj(0))
                pending_proj.append(mk_proj(1))

            def drive():
                NP_PAIRS = wpc // 2
                if ablate == 'attnonly':
                    for p in range(NP_PAIRS):
                        for _ in run_attention(2 * p, fakeqk, [fakevp[0:2], fakevp[2:4]]):
                            pass
                    flush_proj()
                    return
                st = {}
                for _ in qkv_stage(0, st):
                    pass
                attn = None if ablate == 'qkvonly' else run_attention(0, st["qkT"], st["vplus"])
                for p in range(1, NP_PAIRS):
                    st = {}
                    for _ in qkv_stage(p, st):
                        if attn is not None:
                            next(attn, None)
                            next(attn, None)
                    if attn is not None:
                        for _ in attn:
                            pass
                    attn = None if ablate == 'qkvonly' else run_attention(
                        2 * p, st["qkT"], st["vplus"])
                if attn is not None:
                    for _ in attn:
                        pass
                flush_proj()

            if repeat == 1:
                drive()
                win_ctx.close()
            else:
                def rbody(i):
                    drive()
                with tc.For_i(0, repeat, 1) as _:
                    rbody(_)
            win_ctx.close()

    nc.compile()
    return nc


_PROG_CACHE = {}


def _get_prog(wpc, repeat=1, has_qkv_b=True, has_proj_b=True, ablate=None):
    key = (wpc, repeat, has_qkv_b, has_proj_b, ablate)
    if key not in _PROG_CACHE:
        _PROG_CACHE[key] = build_program(wpc, repeat, has_qkv_b, has_proj_b, ablate)
    return _PROG_CACHE[key]


def make_in_maps(inputs, wpc=WPC):
    ident, sigperm, ones = _host_consts()
    x = np.ascontiguousarray(np.asarray(inputs["x"], dtype=np.float32))
    shared = {
        "qkv_w": np.asarray(inputs["qkv_w"], np.float32),
        "qkv_b": np.asarray(inputs["qkv_b"], np.float32),
        "proj_w": np.asarray(inputs["proj_w"], np.float32),
        "proj_b": np.asarray(inputs["proj_b"], np.float32),
        "p1_w": np.asarray(inputs["p1_w"], np.float32),
        "p2_w": np.asarray(inputs["p2_w"], np.float32),
        "p3_w": np.asarray(inputs["p3_w"], np.float32),
        "p1_b": np.asarray(inputs["p1_b"], np.float32),
        "p2_b": np.asarray(inputs["p2_b"], np.float32),
        "p3_b": np.asarray(inputs["p3_b"], np.float32),
        "g1": np.asarray(inputs["g1"], np.float32),
        "g2": np.asarray(inputs["g2"], np.float32),
        "g3": np.asarray(inputs["g3"], np.float32),
        "b1": np.asarray(inputs["b1"], np.float32),
        "b2": np.asarray(inputs["b2"], np.float32),
        "b3": np.asarray(inputs["b3"], np.float32),
        "p4_w": np.asarray(inputs["p4_w"], np.float32),
        "p4_b": np.asarray(inputs["p4_b"], np.float32),
        "biases": np.asarray(inputs["biases"], np.float32),
        "ident": ident,
        "sigperm": sigperm,
        "ones": ones,
    }
    in_maps = []
    for cidx in range(NCORES):
        m = dict(shared)
        m["x"] = x[cidx * wpc:(cidx + 1) * wpc].reshape(wpc * N, C)
        in_maps.append(m)
    return in_maps


def kernel(**inputs):
    has_qkv_b = bool(np.any(np.asarray(inputs["qkv_b"])))
    has_proj_b = bool(np.any(np.asarray(inputs["proj_b"])))
    nc = _get_prog(WPC, 1, has_qkv_b, has_proj_b)
    in_maps = make_in_maps(inputs, WPC)
    res = run_bass_kernel_spmd(nc, in_maps, list(range(NCORES)))
    out = np.concatenate(
        [res.results[c]["y"].reshape(WPC, N, C) for c in range(NCORES)], axis=0
    )
    return out.astype(np.float32)


if __name__ == "__main__":
    rng = np.random.default_rng(0)
    demo = {
        "x": rng.standard_normal((B, N, C), dtype=np.float32),
    }
    print("use test.py for the full check")

